# revision 21
# baseline (speedup 1.0000x reference)
"""GATv2Conv on 8 Trainium2 NeuronCores — edge-parallel, dst-sharded, v3.

v3 (from v2's balanced 600us): rebalance the per-supertile elementwise
pipeline across engines using the Lrelu activation and the Pool engine:

  * leaky(z) is ONE ScalarE Lrelu op (v2 burned abs + a 0.6/0.4 matmul
    split to emulate it); the score transpose-matmul uses l_sb with the
    raw att vector.
  * the z PSUM->SBUF copy moves to the Pool engine (idle in v2), freeing
    ScalarE/DVE; DVE keeps only the xjp multiply and the finalize.

Pipeline per 2048-edge window (dst span < 128), per 512-edge supertile:
  - zb = W_l^T @ xgT + xr-scatter        (2 matmuls, N=512)
  - z_sb = copy(zb) on Pool; l_sb = Lrelu(zb) on ScalarE
  - per 128-edge chunk: zeb[:,t,0:128] = z transpose (identity matmul),
    zeb[:,t,128:132] = l_sb^T @ att  (the GATv2 score, N=4)
  - p = exp(score) edge-major (ScalarE, 16 elems); xjp = z_e * p (DVE)
  - pod[n, 0:128] += S4^T @ xjp, den rides as 4 extra columns
  - finalize: out = pod/den - xr + bias  (softmax weights sum to 1, so
    aggregating z = xl[src] + xr[dst] just shifts the output by xr).
Host concatenates row ranges; zero-degree rows := bias.
"""

import numpy as np
import ml_dtypes

import concourse.bass as bass
import concourse.bacc as bacc
import concourse.mybir as mybir
import concourse.tile as tile
from concourse import library_config

BF16 = ml_dtypes.bfloat16

H, C, F = 4, 32, 128          # heads, channels/head, F = H*C = feature dim
NCORES = 8
P = 128                       # partitions
EPW = 2048                    # edge slots per window
NSUP = EPW // 512             # 4 supertiles of 512 edges
SENTINEL = 999.0              # dst_rel for dummy slots (bf16-safe, != 0..127)
EPS = 1e-16
NEG_SLOPE = 0.2


# ----------------------------------------------------------------- host prep


def _split_cores(sdst, n_nodes, n_edges):
    eb = [0]
    nb = [0]
    for c in range(1, NCORES):
        pos = min(n_edges - 1, (n_edges * c) // NCORES)
        node = int(sdst[pos])
        eb.append(int(np.searchsorted(sdst, node)))
        nb.append(node)
    eb.append(n_edges)
    nb.append(n_nodes)
    return eb, nb


def _build_core_windows(ssrc_c, sdst_c, nb_c, nr_c):
    """Pack one core's dst-sorted edges into 2048-slot windows whose dst
    span is < 128 nodes.  Slot j holds edge j (supertile s = j//512,
    chunk t = (j%512)//128, partition p = j%128)."""
    ne = len(ssrc_c)
    if ne == 0:
        nodes = np.zeros(0, np.int64)
        counts = np.zeros(0, np.int64)
    else:
        change = np.flatnonzero(np.diff(sdst_c)) + 1
        starts = np.concatenate(([0], change))
        nodes = sdst_c[starts]
        counts = np.diff(np.concatenate((starts, [ne])))
    nseg = len(nodes)
    assert nseg == 0 or counts.max() <= EPW

    win_segs = []
    i = 0
    while i < nseg:
        base = nodes[i]
        tot = 0
        j = i
        while j < nseg and nodes[j] - base < P and tot + counts[j] <= EPW:
            tot += counts[j]
            j += 1
        assert j > i
        win_segs.append((i, j))
        i = j
    wc = len(win_segs)

    seg_e0 = np.concatenate(([0], np.cumsum(counts))).astype(np.int64)
    src_slots = np.full((wc, EPW), -1, np.int64)   # -1 -> zero row
    dstrel = np.full((wc, EPW), SENTINEL, np.float32)
    win_nb = np.zeros(wc, np.int64)
    own_end = np.zeros(wc, np.int64)
    for w, (si, sj) in enumerate(win_segs):
        win_nb[w] = nodes[si]
        e0, e1 = int(seg_e0[si]), int(seg_e0[sj])
        cnt = e1 - e0
        src_slots[w, :cnt] = ssrc_c[e0:e1]
        dstrel[w, :cnt] = (sdst_c[e0:e1] - win_nb[w]).astype(np.float32)
        oe = nodes[sj] if sj < nseg else nb_c + nr_c
        own_end[w] = min(oe, win_nb[w] + P)
    return dict(win_nb=win_nb, own_end=own_end, src_slots=src_slots,
                dstrel=dstrel, wc=wc)


# ------------------------------------------------------------- bass program


def _build_nc(W):
    nc = bacc.Bacc("TRN2", target_bir_lowering=False, debug=False)
    bf = mybir.dt.bfloat16
    f32 = mybir.dt.float32

    # window-major-per-partition layouts: a K-window group is one DMA with
    # K*4KB contiguous bytes per partition (big descriptors)
    xgT = nc.dram_tensor("xgT", [P, W * EPW], bf, kind="ExternalInput")
    xwT = nc.dram_tensor("xwT", [P, W * P], bf, kind="ExternalInput")
    # host-shipped one-hot scatter matrices (0/1, exact in fp8)
    f8 = mybir.dt.float8e4
    sts8 = nc.dram_tensor("sts8", [P, W * EPW], f8, kind="ExternalInput")
    s48 = nc.dram_tensor("s48", [P, W * EPW], f8, kind="ExternalInput")
    Wl = nc.dram_tensor("Wl", [P, F], bf, kind="ExternalInput")
    Wr = nc.dram_tensor("Wr", [P, F], bf, kind="ExternalInput")
    # i128 = I (transpose identity); attH[:, h] = att rows (score weights)
    i128 = nc.dram_tensor("i128", [P, F], bf, kind="ExternalInput")
    attH = nc.dram_tensor("attH", [P, H], bf, kind="ExternalInput")
    biasbc = nc.dram_tensor("biasbc", [P, F], bf, kind="ExternalInput")

    outp = nc.dram_tensor("outp", [W * P, F], bf, kind="ExternalOutput")

    # Prelu (parametric_relu) lives in the same activation table as Exp /
    # Copy (exp_and_others) — Lrelu would force 1.3us table swaps per use.
    Lrelu = mybir.ActivationFunctionType.Prelu
    Exp = mybir.ActivationFunctionType.Exp
    sub = mybir.AluOpType.subtract
    mult = mybir.AluOpType.mult
    add = mybir.AluOpType.add

    with tile.TileContext(nc) as tc:
        with tc.tile_pool(name="const", bufs=1) as cpool:
            nc.gpsimd.load_library(library_config.mlp)
            wl_sb = cpool.tile([P, F], bf, tag="wl")
            nc.sync.dma_start(out=wl_sb[:], in_=Wl[:])
            wr_sb = cpool.tile([P, F], bf, tag="wr")
            nc.sync.dma_start(out=wr_sb[:], in_=Wr[:])
            i128_sb = cpool.tile([P, F], bf, tag="i128")
            nc.sync.dma_start(out=i128_sb[:], in_=i128[:])
            attH_sb = cpool.tile([P, H], bf, tag="attH")
            nc.sync.dma_start(out=attH_sb[:], in_=attH[:])
            biasbc_sb = cpool.tile([P, F], bf, tag="biasbc")
            nc.sync.dma_start(out=biasbc_sb[:], in_=biasbc[:])

            # xr (node-major, window-indexed) resident for the whole run
            xrar = cpool.tile([P, W, F], bf, tag="xrar")
            xrb = cpool.tile([P, W, F], bf, tag="xrb")
            # whole-run inputs loaded once (kills per-window small DMAs)
            xwT_sb = cpool.tile([P, W, P], bf, tag="xwT_sb")
            nc.sync.dma_start(
                out=xwT_sb[:],
                in_=xwT.ap().rearrange("p (a b) -> p a b", a=W))

            # ---------------- phase 1: xr = x @ W_r, xrb = bias - xr
            # batched 4 windows per PSUM tile: 1 copy + 1 TT per 4 windows
            with (
                tc.tile_pool(name="ph1", bufs=3) as p1,
                tc.tile_pool(name="ph1ps", bufs=3, space="PSUM") as p1ps,
            ):
                for w4 in range(W // 4):
                    ps = p1ps.tile([P, 4, F], f32, tag="ps1")
                    for k in range(4):
                        nc.tensor.matmul(
                            out=ps[:, k, :], lhsT=xwT_sb[:, 4 * w4 + k, :],
                            rhs=wr_sb[:], start=True, stop=True,
                            skip_group_check=True)
                    wsl = slice(4 * w4, 4 * w4 + 4)
                    nc.vector.tensor_copy(out=xrar[:, wsl, :], in_=ps[:])
                    nc.vector.tensor_tensor(
                        out=xrb[:, wsl, :],
                        in0=biasbc_sb[:, None, :].broadcast_to([P, 4, F]),
                        in1=ps[:], op=sub)

            # ---------------- phase 2: edge windows
            with (
                tc.tile_pool(name="win", bufs=3) as wp,
                tc.tile_pool(name="mid", bufs=3) as mp,
                tc.tile_pool(name="fin", bufs=2) as fp,
                tc.tile_pool(name="pszb", bufs=2, space="PSUM") as ps_zb,
                tc.tile_pool(name="psze", bufs=2, space="PSUM") as ps_ze,
                tc.tile_pool(name="pssc", bufs=2, space="PSUM") as ps_sc,
                tc.tile_pool(name="psod", bufs=2, space="PSUM") as ps_od,
            ):
                assert W % 4 == 0
                for g in range(W // 4):
                    gl = slice(4 * g * EPW, 4 * (g + 1) * EPW)
                    xg4 = wp.tile([P, 4, NSUP, 512], bf, tag="xg4")
                    nc.sync.dma_start(
                        out=xg4[:],
                        in_=xgT[:, gl].rearrange(
                            "p (v a b) -> p v a b", v=4, a=NSUP))
                    sts4 = wp.tile([P, 4, EPW], f8, tag="sts4")
                    nc.sync.dma_start(
                        out=sts4[:],
                        in_=sts8[:, gl].rearrange("p (v e) -> p v e", v=4))
                    s484 = wp.tile([P, 4, 4 * NSUP, P], f8, tag="s484")
                    nc.sync.dma_start(
                        out=s484[:],
                        in_=s48[:, gl].rearrange(
                            "p (v a b) -> p v a b", v=4, a=4 * NSUP))
                    for wi in range(4):
                        w = 4 * g + wi
                        pod = ps_od.tile([P, F + H], f32, tag="pod")
                        # raw scores (16 per supertile) collect in their own
                        # bank: single-shot MMs must not share a bank with
                        # the open pod accumulation group
                        scw = ps_sc.tile([P, 4 * 16], f32, tag="scw")
                        stageb = [None] * NSUP
                        xjp2s = [None] * (NSUP // 2)

                        def pair_stage(q):
                            """exp over the PAIR's 32 raw scores, straight
                            into the den columns of one paired xjp tile;
                            then the two xjp multiplies."""
                            xjp2 = mp.tile([P, 2, 4, F + H], bf, tag="xjp2")
                            xjp2s[q] = xjp2
                            nc.scalar.activation(
                                out=xjp2[:, :, :, F: F + H],
                                in_=scw[:, 32 * q: 32 * q + 32]
                                .rearrange("p (u t h) -> p u t h", u=2, t=4),
                                func=Exp)
                            for h2 in range(2):
                                s = 2 * q + h2
                                nc.vector.tensor_tensor(
                                    out=xjp2[:, h2, :, 0:F]
                                    .rearrange("p t (h c) -> p t h c", h=H),
                                    in0=stageb[s]
                                    .rearrange("p t (h c) -> p t h c", h=H),
                                    in1=xjp2[:, h2, :, F: F + H, None]
                                    .broadcast_to([P, 4, H, C]),
                                    op=mult)

                        def pod_stage(q):
                            for h2 in range(2):
                                s = 2 * q + h2
                                for t in range(4):
                                    first = s == 0 and t == 0
                                    last = s == NSUP - 1 and t == 3
                                    nc.tensor.matmul(
                                        out=pod[:],
                                        lhsT=s484[:, wi, 4 * s + t, :],
                                        rhs=xjp2s[q][:, h2, t, :],
                                        start=first, stop=last)

                        for s in range(NSUP):
                            zb = ps_zb.tile([P, 512], f32, tag="zb")
                            nc.tensor.matmul(
                                out=zb[:], lhsT=wl_sb[:],
                                rhs=xg4[:, wi, s, :],
                                start=True, stop=False)
                            nc.tensor.matmul(
                                out=zb[:], lhsT=xrar[:, w, :],
                                rhs=sts4[:, wi, 512 * s: 512 * (s + 1)],
                                start=False, stop=True)
                            # raw z for the transpose/aggregation and
                            # leaky(z) for the score; GPSIMD cannot read
                            # PSUM (and its Pool ISA lacks tensor_tensor),
                            # so split the copies Scalar/DVE
                            z_sb = mp.tile([P, 512], bf, tag="z_sb")
                            if s % 2 == 0:
                                nc.scalar.copy(out=z_sb[:], in_=zb[:])
                            else:
                                nc.vector.tensor_copy(out=z_sb[:], in_=zb[:])
                            l_sb = mp.tile([P, 512], bf, tag="l_sb")
                            nc.scalar.activation(out=l_sb[:], in_=zb[:],
                                                 func=Lrelu, alpha=NEG_SLOPE)

                            # zeb holds the transposed z only (1 PSUM bank);
                            # raw scores land in the od bank cols 132+
                            zeb = ps_ze.tile([P, 4, F], f32, tag="zeb")
                            for t in range(4):
                                nc.tensor.matmul(
                                    out=zeb[:, t, :],
                                    lhsT=z_sb[:, P * t: P * (t + 1)],
                                    rhs=i128_sb[:], start=True, stop=True,
                                    skip_group_check=True)
                                so = 16 * s + 4 * t
                                nc.tensor.matmul(
                                    out=scw[:, so: so + 4],
                                    lhsT=l_sb[:, P * t: P * (t + 1)],
                                    rhs=attH_sb[:], start=True, stop=True,
                                    skip_group_check=True)
                            stageb[s] = zeb
                            if s % 2 == 1:
                                pair_stage(s // 2)
                                pod_stage(s // 2)

                        # finalize: rd = 1/den (den>0 wherever this window
                        # owns an edge; deg==0 rows are overwritten with
                        # bias on the host, so no EPS guard is needed);
                        # out_h = pod_h * rd_h + (bias - xr)_h  per head
                        # rd = 1/den; den>0 wherever this window owns an
                        # edge, and deg==0 rows (1/0 -> inf -> NaN) are
                        # overwritten with bias on the host, so no EPS.
                        # out_h = pod_h * rd_h + (bias - xr)_h  per head.
                        rd = fp.tile([P, H], f32, tag="rd")
                        nc.vector.reciprocal(out=rd[:], in_=pod[:, F: F + H])
                        fin2 = fp.tile([P, F], bf, tag="fin2")
                        for h in range(H):
                            nc.vector.scalar_tensor_tensor(
                                out=fin2[:, C * h: C * (h + 1)],
                                in0=pod[:, C * h: C * (h + 1)],
                                scalar=rd[:, h: h + 1],
                                in1=xrb[:, w, C * h: C * (h + 1)],
                                op0=mult, op1=add)
                        nc.sync.dma_start(
                            out=outp[P * w: P * (w + 1), :], in_=fin2[:])

    nc.compile()
    return nc


# ------------------------------------------------------------------- driver


def _prepare(x, edge_index, W_l, W_r, att, bias):
    n_nodes = x.shape[0]
    n_edges = edge_index.shape[1]
    src = np.asarray(edge_index[0], np.int64)
    dst = np.asarray(edge_index[1], np.int64)
    order = np.argsort(dst, kind="stable")
    ssrc = src[order]
    sdst = dst[order]

    eb, nb = _split_cores(sdst, n_nodes, n_edges)
    cores = []
    for c in range(NCORES):
        nr_c = nb[c + 1] - nb[c]
        cores.append(_build_core_windows(
            ssrc[eb[c]:eb[c + 1]], sdst[eb[c]:eb[c + 1]], nb[c], nr_c))
    W = (max(cd["wc"] for cd in cores) + 3) // 4 * 4  # group-of-4 windows

    xb = np.asarray(x, np.float32).astype(BF16)
    # row n_nodes = zeros for dummy slots
    xpad = np.vstack([xb, np.zeros((1, F), BF16)])

    wl_np = np.asarray(W_l, np.float32).astype(BF16)
    wr_np = np.asarray(W_r, np.float32).astype(BF16)
    att_np = np.asarray(att, np.float32)
    i128_np = np.eye(P, dtype=np.float32)
    attH_np = np.zeros((P, H), np.float32)
    for h in range(H):
        attH_np[C * h:C * (h + 1), h] = att_np[h]
    bias_np = np.asarray(bias, np.float32)
    biasbc_np = np.tile(bias_np[None, :], (P, 1))

    in_maps = []
    for c in range(NCORES):
        cd = cores[c]
        wc = cd["wc"]

        slots = np.full((W, EPW), n_nodes, np.int64)
        slots[:wc] = np.where(cd["src_slots"] >= 0, cd["src_slots"], n_nodes)
        # xgT[:, w*EPW + j] = xpad[slots[w, j]] (window-major per partition)
        xgT_np = np.ascontiguousarray(
            xpad[slots.reshape(-1)].reshape(W, EPW, P)
            .transpose(2, 0, 1).reshape(P, W * EPW))

        F8 = ml_dtypes.float8_e4m3fn
        iota = np.arange(P, dtype=np.float32)
        # sts8[n, w*EPW + e] = (dstrel[w, e] == n)
        sts8_np = np.zeros((W, P, EPW), F8)
        sts8_np[:wc] = (cd["dstrel"][:, None, :] ==
                        iota[None, :, None]).astype(F8)
        sts8_np = np.ascontiguousarray(
            sts8_np.transpose(1, 0, 2).reshape(P, W * EPW))
        # s48[p, w*EPW + (4s+t)*P + n] = (dstrel[w, 512s+128t+p] == n)
        drel_c = cd["dstrel"].reshape(wc, 4 * NSUP, P).transpose(0, 2, 1)
        s48_np = np.zeros((W, P, EPW), F8)
        s48_np[:wc] = (drel_c[:, :, :, None] ==
                       iota[None, None, None, :]).astype(F8).reshape(
                           wc, P, EPW)
        s48_np = np.ascontiguousarray(
            s48_np.transpose(1, 0, 2).reshape(P, W * EPW))

        # xwT[:, P*w + p] = x[win_nb[w] + p] (zero outside range)
        xwT_np = np.zeros((P, W * P), BF16)
        for w in range(wc):
            lo = int(cd["win_nb"][w])
            hi = min(lo + P, n_nodes)
            xwT_np[:, P * w: P * w + (hi - lo)] = xb[lo:hi].T

        in_maps.append({
            "xgT": xgT_np.astype(BF16),
            "xwT": xwT_np,
            "sts8": sts8_np,
            "s48": s48_np,
            "Wl": wl_np,
            "Wr": wr_np,
            "i128": i128_np.astype(BF16),
            "attH": attH_np.astype(BF16),
            "biasbc": biasbc_np.astype(BF16),
        })

    meta = dict(W=W, nb=nb, n_nodes=n_nodes, bias=bias_np, cores=cores,
                deg=np.bincount(dst, minlength=n_nodes))
    return in_maps, meta


_last_results = None


def kernel(x, edge_index, W_l, W_r, att, bias, _sim=False, _trace=False):
    global _last_results
    in_maps, meta = _prepare(x, edge_index, W_l, W_r, att, bias)
    nc = _build_nc(meta["W"])

    if _sim:
        from concourse.bass_interp import CoreSim
        results = []
        for c in range(NCORES):
            # inf/NaN are expected on zero-degree rows (1/den with den=0);
            # the host overwrites them with bias
            sim = CoreSim(nc, trace=False,
                          require_finite=False, require_nnan=False)
            for k, v in in_maps[c].items():
                sim.tensor(k)[:] = v
            sim.tensor("outp")[:] = 0.0
            sim.simulate()
            results.append({"outp": np.array(sim.tensor("outp"))})
    else:
        from concourse import bass_utils
        r = bass_utils.run_bass_kernel_spmd(
            nc, in_maps, core_ids=list(range(NCORES)), trace=_trace)
        _last_results = r
        results = r.results

    n_nodes = meta["n_nodes"]
    out = np.empty((n_nodes, F), np.float32)
    for c in range(NCORES):
        cd = meta["cores"][c]
        st = results[c]["outp"].astype(np.float32)
        for w in range(cd["wc"]):
            lo = int(cd["win_nb"][w])
            hi = int(cd["own_end"][w])
            out[lo:hi] = st[P * w: P * w + (hi - lo)]
    out[meta["deg"] == 0] = meta["bias"][None, :]
    return out


# revision 22
# speedup vs baseline: 1.0158x; 1.0158x over previous
"""GATv2Conv on 8 Trainium2 NeuronCores — edge-parallel, dst-sharded, v3.

v3 (from v2's balanced 600us): rebalance the per-supertile elementwise
pipeline across engines using the Lrelu activation and the Pool engine:

  * leaky(z) is ONE ScalarE Lrelu op (v2 burned abs + a 0.6/0.4 matmul
    split to emulate it); the score transpose-matmul uses l_sb with the
    raw att vector.
  * the z PSUM->SBUF copy moves to the Pool engine (idle in v2), freeing
    ScalarE/DVE; DVE keeps only the xjp multiply and the finalize.

Pipeline per 2048-edge window (dst span < 128), per 512-edge supertile:
  - zb = W_l^T @ xgT + xr-scatter        (2 matmuls, N=512)
  - z_sb = copy(zb) on Pool; l_sb = Lrelu(zb) on ScalarE
  - per 128-edge chunk: zeb[:,t,0:128] = z transpose (identity matmul),
    zeb[:,t,128:132] = l_sb^T @ att  (the GATv2 score, N=4)
  - p = exp(score) edge-major (ScalarE, 16 elems); xjp = z_e * p (DVE)
  - pod[n, 0:128] += S4^T @ xjp, den rides as 4 extra columns
  - finalize: out = pod/den - xr + bias  (softmax weights sum to 1, so
    aggregating z = xl[src] + xr[dst] just shifts the output by xr).
Host concatenates row ranges; zero-degree rows := bias.
"""

import numpy as np
import ml_dtypes

import concourse.bass as bass
import concourse.bacc as bacc
import concourse.mybir as mybir
import concourse.tile as tile
from concourse import library_config

BF16 = ml_dtypes.bfloat16

H, C, F = 4, 32, 128          # heads, channels/head, F = H*C = feature dim
NCORES = 8
P = 128                       # partitions
EPW = 2048                    # edge slots per window
NSUP = EPW // 512             # 4 supertiles of 512 edges
SENTINEL = 999.0              # dst_rel for dummy slots (bf16-safe, != 0..127)
EPS = 1e-16
NEG_SLOPE = 0.2


# ----------------------------------------------------------------- host prep


def _split_cores(sdst, n_nodes, n_edges):
    eb = [0]
    nb = [0]
    for c in range(1, NCORES):
        pos = min(n_edges - 1, (n_edges * c) // NCORES)
        node = int(sdst[pos])
        eb.append(int(np.searchsorted(sdst, node)))
        nb.append(node)
    eb.append(n_edges)
    nb.append(n_nodes)
    return eb, nb


def _build_core_windows(ssrc_c, sdst_c, nb_c, nr_c):
    """Pack one core's dst-sorted edges into 2048-slot windows whose dst
    span is < 128 nodes.  Slot j holds edge j (supertile s = j//512,
    chunk t = (j%512)//128, partition p = j%128)."""
    ne = len(ssrc_c)
    if ne == 0:
        nodes = np.zeros(0, np.int64)
        counts = np.zeros(0, np.int64)
    else:
        change = np.flatnonzero(np.diff(sdst_c)) + 1
        starts = np.concatenate(([0], change))
        nodes = sdst_c[starts]
        counts = np.diff(np.concatenate((starts, [ne])))
    nseg = len(nodes)
    assert nseg == 0 or counts.max() <= EPW

    win_segs = []
    i = 0
    while i < nseg:
        base = nodes[i]
        tot = 0
        j = i
        while j < nseg and nodes[j] - base < P and tot + counts[j] <= EPW:
            tot += counts[j]
            j += 1
        assert j > i
        win_segs.append((i, j))
        i = j
    wc = len(win_segs)

    seg_e0 = np.concatenate(([0], np.cumsum(counts))).astype(np.int64)
    src_slots = np.full((wc, EPW), -1, np.int64)   # -1 -> zero row
    dstrel = np.full((wc, EPW), SENTINEL, np.float32)
    win_nb = np.zeros(wc, np.int64)
    own_end = np.zeros(wc, np.int64)
    for w, (si, sj) in enumerate(win_segs):
        win_nb[w] = nodes[si]
        e0, e1 = int(seg_e0[si]), int(seg_e0[sj])
        cnt = e1 - e0
        src_slots[w, :cnt] = ssrc_c[e0:e1]
        dstrel[w, :cnt] = (sdst_c[e0:e1] - win_nb[w]).astype(np.float32)
        oe = nodes[sj] if sj < nseg else nb_c + nr_c
        own_end[w] = min(oe, win_nb[w] + P)
    return dict(win_nb=win_nb, own_end=own_end, src_slots=src_slots,
                dstrel=dstrel, wc=wc)


# ------------------------------------------------------------- bass program


def _build_nc(W):
    nc = bacc.Bacc("TRN2", target_bir_lowering=False, debug=False)
    bf = mybir.dt.bfloat16
    f32 = mybir.dt.float32

    # window-major-per-partition layouts: a K-window group is one DMA with
    # K*4KB contiguous bytes per partition (big descriptors)
    xgT = nc.dram_tensor("xgT", [P, W * EPW], bf, kind="ExternalInput")
    xwT = nc.dram_tensor("xwT", [P, W * P], bf, kind="ExternalInput")
    # host-shipped one-hot scatter matrices (0/1, exact in fp8)
    f8 = mybir.dt.float8e4
    sts8 = nc.dram_tensor("sts8", [P, W * EPW], f8, kind="ExternalInput")
    s48 = nc.dram_tensor("s48", [P, W * EPW], f8, kind="ExternalInput")
    Wl = nc.dram_tensor("Wl", [P, F], bf, kind="ExternalInput")
    Wr = nc.dram_tensor("Wr", [P, F], bf, kind="ExternalInput")
    # i128 = I (transpose identity); attH[:, h] = att rows (score weights)
    i128 = nc.dram_tensor("i128", [P, F], bf, kind="ExternalInput")
    attH = nc.dram_tensor("attH", [P, H], bf, kind="ExternalInput")
    biasbc = nc.dram_tensor("biasbc", [P, F], bf, kind="ExternalInput")

    outp = nc.dram_tensor("outp", [W * P, F], bf, kind="ExternalOutput")

    # Prelu (parametric_relu) lives in the same activation table as Exp /
    # Copy (exp_and_others) — Lrelu would force 1.3us table swaps per use.
    Lrelu = mybir.ActivationFunctionType.Prelu
    Exp = mybir.ActivationFunctionType.Exp
    sub = mybir.AluOpType.subtract
    mult = mybir.AluOpType.mult
    add = mybir.AluOpType.add

    with tile.TileContext(nc) as tc:
        with tc.tile_pool(name="const", bufs=1) as cpool:
            nc.gpsimd.load_library(library_config.mlp)
            wl_sb = cpool.tile([P, F], bf, tag="wl")
            nc.sync.dma_start(out=wl_sb[:], in_=Wl[:])
            wr_sb = cpool.tile([P, F], bf, tag="wr")
            nc.sync.dma_start(out=wr_sb[:], in_=Wr[:])
            i128_sb = cpool.tile([P, F], bf, tag="i128")
            nc.sync.dma_start(out=i128_sb[:], in_=i128[:])
            attH_sb = cpool.tile([P, H], bf, tag="attH")
            nc.sync.dma_start(out=attH_sb[:], in_=attH[:])
            biasbc_sb = cpool.tile([P, F], bf, tag="biasbc")
            nc.sync.dma_start(out=biasbc_sb[:], in_=biasbc[:])

            # xr (node-major, window-indexed) resident for the whole run
            xrar = cpool.tile([P, W, F], bf, tag="xrar")
            xrb = cpool.tile([P, W, F], bf, tag="xrb")
            # whole-run inputs loaded once (kills per-window small DMAs)
            xwT_sb = cpool.tile([P, W, P], bf, tag="xwT_sb")
            nc.sync.dma_start(
                out=xwT_sb[:],
                in_=xwT.ap().rearrange("p (a b) -> p a b", a=W))

            # ---------------- phase 1: xr = x @ W_r, xrb = bias - xr
            # batched 4 windows per PSUM tile: 1 copy + 1 TT per 4 windows
            with (
                tc.tile_pool(name="ph1", bufs=3) as p1,
                tc.tile_pool(name="ph1ps", bufs=3, space="PSUM") as p1ps,
            ):
                for w4 in range(W // 4):
                    ps = p1ps.tile([P, 4, F], f32, tag="ps1")
                    for k in range(4):
                        nc.tensor.matmul(
                            out=ps[:, k, :], lhsT=xwT_sb[:, 4 * w4 + k, :],
                            rhs=wr_sb[:], start=True, stop=True,
                            skip_group_check=True)
                    wsl = slice(4 * w4, 4 * w4 + 4)
                    nc.vector.tensor_copy(out=xrar[:, wsl, :], in_=ps[:])
                    nc.vector.tensor_tensor(
                        out=xrb[:, wsl, :],
                        in0=biasbc_sb[:, None, :].broadcast_to([P, 4, F]),
                        in1=ps[:], op=sub)

            # ---------------- phase 2: edge windows
            with (
                tc.tile_pool(name="win", bufs=3) as wp,
                tc.tile_pool(name="mid", bufs=3) as mp,
                tc.tile_pool(name="fin", bufs=2) as fp,
                tc.tile_pool(name="pszb", bufs=2, space="PSUM") as ps_zb,
                tc.tile_pool(name="psze", bufs=3, space="PSUM") as ps_ze,
                tc.tile_pool(name="pssc", bufs=1, space="PSUM") as ps_sc,
                tc.tile_pool(name="psod", bufs=2, space="PSUM") as ps_od,
            ):
                assert W % 4 == 0
                for g in range(W // 4):
                    gl = slice(4 * g * EPW, 4 * (g + 1) * EPW)
                    xg4 = wp.tile([P, 4, NSUP, 512], bf, tag="xg4")
                    nc.sync.dma_start(
                        out=xg4[:],
                        in_=xgT[:, gl].rearrange(
                            "p (v a b) -> p v a b", v=4, a=NSUP))
                    sts4 = wp.tile([P, 4, EPW], f8, tag="sts4")
                    nc.sync.dma_start(
                        out=sts4[:],
                        in_=sts8[:, gl].rearrange("p (v e) -> p v e", v=4))
                    s484 = wp.tile([P, 4, 4 * NSUP, P], f8, tag="s484")
                    nc.sync.dma_start(
                        out=s484[:],
                        in_=s48[:, gl].rearrange(
                            "p (v a b) -> p v a b", v=4, a=4 * NSUP))
                    for wi in range(4):
                        w = 4 * g + wi
                        pod = ps_od.tile([P, F + H], f32, tag="pod")
                        # raw scores (16 per supertile) collect in their own
                        # bank: single-shot MMs must not share a bank with
                        # the open pod accumulation group
                        scw = ps_sc.tile([P, 4 * 16], f32, tag="scw")
                        stageb = [None] * NSUP
                        xjp2s = [None] * (NSUP // 2)

                        def pair_stage(q):
                            """exp over the PAIR's 32 raw scores, straight
                            into the den columns of one paired xjp tile;
                            then the two xjp multiplies."""
                            xjp2 = mp.tile([P, 2, 4, F + H], bf, tag="xjp2")
                            xjp2s[q] = xjp2
                            nc.scalar.activation(
                                out=xjp2[:, :, :, F: F + H],
                                in_=scw[:, 32 * q: 32 * q + 32]
                                .rearrange("p (u t h) -> p u t h", u=2, t=4),
                                func=Exp)
                            for h2 in range(2):
                                s = 2 * q + h2
                                nc.vector.tensor_tensor(
                                    out=xjp2[:, h2, :, 0:F]
                                    .rearrange("p t (h c) -> p t h c", h=H),
                                    in0=stageb[s]
                                    .rearrange("p t (h c) -> p t h c", h=H),
                                    in1=xjp2[:, h2, :, F: F + H, None]
                                    .broadcast_to([P, 4, H, C]),
                                    op=mult)

                        def pod_stage(q):
                            for h2 in range(2):
                                s = 2 * q + h2
                                for t in range(4):
                                    first = s == 0 and t == 0
                                    last = s == NSUP - 1 and t == 3
                                    nc.tensor.matmul(
                                        out=pod[:],
                                        lhsT=s484[:, wi, 4 * s + t, :],
                                        rhs=xjp2s[q][:, h2, t, :],
                                        start=first, stop=last)

                        for s in range(NSUP):
                            zb = ps_zb.tile([P, 512], f32, tag="zb")
                            nc.tensor.matmul(
                                out=zb[:], lhsT=wl_sb[:],
                                rhs=xg4[:, wi, s, :],
                                start=True, stop=False)
                            nc.tensor.matmul(
                                out=zb[:], lhsT=xrar[:, w, :],
                                rhs=sts4[:, wi, 512 * s: 512 * (s + 1)],
                                start=False, stop=True)
                            # raw z for the transpose/aggregation and
                            # leaky(z) for the score; GPSIMD cannot read
                            # PSUM (and its Pool ISA lacks tensor_tensor),
                            # so split the copies Scalar/DVE
                            z_sb = mp.tile([P, 512], bf, tag="z_sb")
                            if s % 2 == 0:
                                nc.scalar.copy(out=z_sb[:], in_=zb[:])
                            else:
                                nc.vector.tensor_copy(out=z_sb[:], in_=zb[:])
                            l_sb = mp.tile([P, 512], bf, tag="l_sb")
                            nc.scalar.activation(out=l_sb[:], in_=zb[:],
                                                 func=Lrelu, alpha=NEG_SLOPE)

                            # zeb holds the transposed z only (1 PSUM bank);
                            # raw scores land in the od bank cols 132+
                            zeb = ps_ze.tile([P, 4, F], f32, tag="zeb")
                            for t in range(4):
                                nc.tensor.matmul(
                                    out=zeb[:, t, :],
                                    lhsT=z_sb[:, P * t: P * (t + 1)],
                                    rhs=i128_sb[:], start=True, stop=True,
                                    skip_group_check=True)
                                so = 16 * s + 4 * t
                                nc.tensor.matmul(
                                    out=scw[:, so: so + 4],
                                    lhsT=l_sb[:, P * t: P * (t + 1)],
                                    rhs=attH_sb[:], start=True, stop=True,
                                    skip_group_check=True)
                            stageb[s] = zeb
                            if s % 2 == 1:
                                pair_stage(s // 2)
                                pod_stage(s // 2)

                        # finalize: rd = 1/den (den>0 wherever this window
                        # owns an edge; deg==0 rows are overwritten with
                        # bias on the host, so no EPS guard is needed);
                        # out_h = pod_h * rd_h + (bias - xr)_h  per head
                        # rd = 1/den; den>0 wherever this window owns an
                        # edge, and deg==0 rows (1/0 -> inf -> NaN) are
                        # overwritten with bias on the host, so no EPS.
                        # out_h = pod_h * rd_h + (bias - xr)_h  per head.
                        rd = fp.tile([P, H], f32, tag="rd")
                        nc.vector.reciprocal(out=rd[:], in_=pod[:, F: F + H])
                        fin2 = fp.tile([P, F], bf, tag="fin2")
                        for h in range(H):
                            nc.vector.scalar_tensor_tensor(
                                out=fin2[:, C * h: C * (h + 1)],
                                in0=pod[:, C * h: C * (h + 1)],
                                scalar=rd[:, h: h + 1],
                                in1=xrb[:, w, C * h: C * (h + 1)],
                                op0=mult, op1=add)
                        nc.sync.dma_start(
                            out=outp[P * w: P * (w + 1), :], in_=fin2[:])

    nc.compile()
    return nc


# ------------------------------------------------------------------- driver


def _prepare(x, edge_index, W_l, W_r, att, bias):
    n_nodes = x.shape[0]
    n_edges = edge_index.shape[1]
    src = np.asarray(edge_index[0], np.int64)
    dst = np.asarray(edge_index[1], np.int64)
    order = np.argsort(dst, kind="stable")
    ssrc = src[order]
    sdst = dst[order]

    eb, nb = _split_cores(sdst, n_nodes, n_edges)
    cores = []
    for c in range(NCORES):
        nr_c = nb[c + 1] - nb[c]
        cores.append(_build_core_windows(
            ssrc[eb[c]:eb[c + 1]], sdst[eb[c]:eb[c + 1]], nb[c], nr_c))
    W = (max(cd["wc"] for cd in cores) + 3) // 4 * 4  # group-of-4 windows

    xb = np.asarray(x, np.float32).astype(BF16)
    # row n_nodes = zeros for dummy slots
    xpad = np.vstack([xb, np.zeros((1, F), BF16)])

    wl_np = np.asarray(W_l, np.float32).astype(BF16)
    wr_np = np.asarray(W_r, np.float32).astype(BF16)
    att_np = np.asarray(att, np.float32)
    i128_np = np.eye(P, dtype=np.float32)
    attH_np = np.zeros((P, H), np.float32)
    for h in range(H):
        attH_np[C * h:C * (h + 1), h] = att_np[h]
    bias_np = np.asarray(bias, np.float32)
    biasbc_np = np.tile(bias_np[None, :], (P, 1))

    in_maps = []
    for c in range(NCORES):
        cd = cores[c]
        wc = cd["wc"]

        slots = np.full((W, EPW), n_nodes, np.int64)
        slots[:wc] = np.where(cd["src_slots"] >= 0, cd["src_slots"], n_nodes)
        # xgT[:, w*EPW + j] = xpad[slots[w, j]] (window-major per partition)
        xgT_np = np.ascontiguousarray(
            xpad[slots.reshape(-1)].reshape(W, EPW, P)
            .transpose(2, 0, 1).reshape(P, W * EPW))

        F8 = ml_dtypes.float8_e4m3fn
        iota = np.arange(P, dtype=np.float32)
        # sts8[n, w*EPW + e] = (dstrel[w, e] == n)
        sts8_np = np.zeros((W, P, EPW), F8)
        sts8_np[:wc] = (cd["dstrel"][:, None, :] ==
                        iota[None, :, None]).astype(F8)
        sts8_np = np.ascontiguousarray(
            sts8_np.transpose(1, 0, 2).reshape(P, W * EPW))
        # s48[p, w*EPW + (4s+t)*P + n] = (dstrel[w, 512s+128t+p] == n)
        drel_c = cd["dstrel"].reshape(wc, 4 * NSUP, P).transpose(0, 2, 1)
        s48_np = np.zeros((W, P, EPW), F8)
        s48_np[:wc] = (drel_c[:, :, :, None] ==
                       iota[None, None, None, :]).astype(F8).reshape(
                           wc, P, EPW)
        s48_np = np.ascontiguousarray(
            s48_np.transpose(1, 0, 2).reshape(P, W * EPW))

        # xwT[:, P*w + p] = x[win_nb[w] + p] (zero outside range)
        xwT_np = np.zeros((P, W * P), BF16)
        for w in range(wc):
            lo = int(cd["win_nb"][w])
            hi = min(lo + P, n_nodes)
            xwT_np[:, P * w: P * w + (hi - lo)] = xb[lo:hi].T

        in_maps.append({
            "xgT": xgT_np.astype(BF16),
            "xwT": xwT_np,
            "sts8": sts8_np,
            "s48": s48_np,
            "Wl": wl_np,
            "Wr": wr_np,
            "i128": i128_np.astype(BF16),
            "attH": attH_np.astype(BF16),
            "biasbc": biasbc_np.astype(BF16),
        })

    meta = dict(W=W, nb=nb, n_nodes=n_nodes, bias=bias_np, cores=cores,
                deg=np.bincount(dst, minlength=n_nodes))
    return in_maps, meta


_last_results = None


def kernel(x, edge_index, W_l, W_r, att, bias, _sim=False, _trace=False):
    global _last_results
    in_maps, meta = _prepare(x, edge_index, W_l, W_r, att, bias)
    nc = _build_nc(meta["W"])

    if _sim:
        from concourse.bass_interp import CoreSim
        results = []
        for c in range(NCORES):
            # inf/NaN are expected on zero-degree rows (1/den with den=0);
            # the host overwrites them with bias
            sim = CoreSim(nc, trace=False,
                          require_finite=False, require_nnan=False)
            for k, v in in_maps[c].items():
                sim.tensor(k)[:] = v
            sim.tensor("outp")[:] = 0.0
            sim.simulate()
            results.append({"outp": np.array(sim.tensor("outp"))})
    else:
        from concourse import bass_utils
        r = bass_utils.run_bass_kernel_spmd(
            nc, in_maps, core_ids=list(range(NCORES)), trace=_trace)
        _last_results = r
        results = r.results

    n_nodes = meta["n_nodes"]
    out = np.empty((n_nodes, F), np.float32)
    for c in range(NCORES):
        cd = meta["cores"][c]
        st = results[c]["outp"].astype(np.float32)
        for w in range(cd["wc"]):
            lo = int(cd["win_nb"][w])
            hi = int(cd["own_end"][w])
            out[lo:hi] = st[P * w: P * w + (hi - lo)]
    out[meta["deg"] == 0] = meta["bias"][None, :]
    return out
